# revision 10
# baseline (speedup 1.0000x reference)
"""Trainium2 Bass kernel for nn_CausalSelfAttention_88364657148333.

8-core SPMD: tensor-parallel over heads (2 q-heads + 1 kv-head per core),
AllToAll reshard before the output projection (each core finishes 1/8 of
the tokens). All matmuls run in float32r (full-rate fp32 on the PE).
"""

import math
import sys

import numpy as np

try:
    import concourse.bass  # noqa: F401
except ImportError:
    sys.path.insert(0, "/opt/trn_rl_repo")

import concourse.bass_isa as bass_isa  # noqa: E402
import concourse.mybir as mybir  # noqa: E402
import concourse.tile as tile  # noqa: E402
from concourse import bacc  # noqa: E402
from concourse.bass_utils import run_bass_kernel_spmd  # noqa: E402

F32 = mybir.dt.float32
F32R = mybir.dt.float32r
AF = mybir.ActivationFunctionType
ALU = mybir.AluOpType
ROP = bass_isa.ReduceOp

B = 2
DIM = 2048
H = 16
HKV = 4
D = 128
NCORES = 8
EPS = 1.1920929e-07
ROPE_DIMS = 64
ROPE_BASE = 10000.0

_BUILD_CACHE: dict = {}


def build(S: int = 2048):
    """Build the SPMD Bass program (identical on all 8 cores)."""
    N = B * S                 # total tokens
    TSLICE = N // NCORES      # output tokens per core
    TB = 256                  # projection token block
    NTB = N // TB
    QB = min(512, S)          # attention query block (within one batch)
    NQB = S // QB
    NT = TSLICE // 128        # out-proj token chunks per core
    NEC = DIM // 512          # out-proj feature chunks
    ROWS = 2 * D + 2          # A2A bundle rows per core (yT h0, yT h1, den)
    assert S % TB == 0 and TB <= S and TSLICE % 128 == 0 and QB % TSLICE == 0

    nc = bacc.Bacc("TRN2", target_bir_lowering=False, num_devices=NCORES)

    # ---- DRAM I/O (per-core contents prepared by kernel() on the host) ----
    xT = nc.dram_tensor("xT", [DIM, N], F32, kind="ExternalInput")
    wall = nc.dram_tensor("wall", [DIM, 514], F32, kind="ExternalInput")
    wprojT = nc.dram_tensor("wprojT", [DIM, DIM], F32, kind="ExternalInput")
    gwT = nc.dram_tensor("gwT", [DIM, H], F32, kind="ExternalInput")
    v0c = nc.dram_tensor("v0c", [N, D], F32, kind="ExternalInput")
    xTg = nc.dram_tensor("xTg", [DIM, TSLICE], F32, kind="ExternalInput")
    cc = nc.dram_tensor("cc", [ROPE_DIMS, S], F32, kind="ExternalInput")
    ss = nc.dram_tensor("ss", [ROPE_DIMS, S], F32, kind="ExternalInput")
    mtri = nc.dram_tensor("mtri", [128, 128], F32, kind="ExternalInput")
    ident = nc.dram_tensor("ident", [128, 128], F32, kind="ExternalInput")
    scal = nc.dram_tensor("scal", [4 + H], F32, kind="ExternalInput")

    outslice = nc.dram_tensor("outslice", [TSLICE, DIM], F32, kind="ExternalOutput")
    rawvT = nc.dram_tensor("rawvT", [D, N], F32, kind="ExternalOutput")

    xT_r = xT.ap().rearrange("(c p) n -> p c n", p=128)        # [128,16,N]
    wall_r = wall.ap().rearrange("(c p) m -> p c m", p=128)    # [128,16,514]
    wprojT_r = wprojT.ap().rearrange("(c p) e -> p c e", p=128)
    gwT_r = gwT.ap().rearrange("(c p) h -> p c h", p=128)
    gxT_r = xTg.ap().rearrange("(c p) n -> p c n", p=128)

    with tile.TileContext(nc) as tc:
        with (
            tc.tile_pool(name="consts", bufs=1) as consts,
            tc.tile_pool(name="dram", bufs=1, space="DRAM") as dram,
        ):
            # ---------- small constants ----------
            mtri_sb = consts.tile([128, 128], F32)
            id_sb = consts.tile([128, 128], F32)
            gwT_sb = consts.tile([128, 16, H], F32R)
            scal_row = consts.tile([1, 4 + H], F32)
            scal_b = consts.tile([128, 4], F32)
            qgs = consts.tile([128, 2], F32)
            gb_sb = consts.tile([H, 1], F32)
            eps_sb = consts.tile([128, 1], F32)
            nc.vector.memset(eps_sb[:], EPS)
            nc.sync.dma_start(mtri_sb[:], mtri[:, :])
            nc.sync.dma_start(id_sb[:], ident[:, :])
            nc.sync.dma_start(gwT_sb[:], gwT_r.bitcast(F32R))
            nc.sync.dma_start(scal_row[:], scal[None, :])
            nc.sync.dma_start(gb_sb[:], scal[4:4 + H, None])
            nc.gpsimd.partition_broadcast(scal_b[:], scal_row[:, 0:4], 128)
            nc.vector.tensor_scalar_mul(qgs[:], scal_b[:, 0:2], 1.0 / math.sqrt(D))
            lam0 = scal_b[:, 2:3]
            lam1 = scal_b[:, 3:4]

            # ---------- A2A bundle buffers (DRAM) ----------
            agi = dram.tile([NCORES * ROWS, TSLICE], F32)
            ago = dram.tile([NCORES * ROWS, TSLICE], F32)
            agi_r = agi[:].rearrange("(j r) c -> r j c", j=NCORES)  # [ROWS,8,TS]

            # ---------- P1+P2 persistent activations ----------
            with tc.tile_pool(name="persist2", bufs=1) as persist2:
                qT = [persist2.tile([128, N], F32R, name=f"qT{h}") for h in range(2)]
                kT = persist2.tile([128, N], F32R)
                vmix = persist2.tile([128, N], F32R)

                # ================= P1: projections =================
                with (
                    tc.tile_pool(name="p1", bufs=2) as p1,
                    tc.tile_pool(name="p1w", bufs=1) as p1w,
                    tc.tile_pool(name="ps1", bufs=3, space="PSUM") as ps1,
                    tc.tile_pool(name="ps1t", bufs=2, space="PSUM") as ps1t,
                ):
                    wall_sb = p1w.tile([128, 16, 514], F32R)
                    cc_sb = p1w.tile([ROPE_DIMS, S], F32)
                    ss_sb = p1w.tile([ROPE_DIMS, S], F32)
                    nc.sync.dma_start(wall_sb[:], wall_r.bitcast(F32R))
                    nc.sync.dma_start(cc_sb[:], cc[:, :])
                    nc.sync.dma_start(ss_sb[:], ss[:, :])
                    for tb in range(NTB):
                        tok0 = tb * TB
                        colr = slice(tok0, tok0 + TB)
                        posr = slice(tok0 % S, tok0 % S + TB)
                        xt = p1.tile([128, 16, TB], F32R, tag="xt", bufs=2)
                        nc.sync.dma_start(xt[:], xT_r[:, :, colr].bitcast(F32R))
                        for g in range(4):
                            ps = ps1.tile([128, TB], F32, tag="proj")
                            for dc in range(16):
                                nc.tensor.matmul(
                                    ps[:],
                                    wall_sb[:, dc, g * 128:(g + 1) * 128],
                                    xt[:, dc, :],
                                    start=(dc == 0),
                                    stop=(dc == 15),
                                )
                            if g < 3:
                                # ---- RMS norm + partial RoPE (q0, q1, k) ----
                                dest = qT[g] if g < 2 else kT
                                sq = p1.tile([128, TB], F32, tag="sq")
                                nc.scalar.square(sq[:], ps[:])
                                ssq = p1.tile([128, TB], F32, tag="ssq")
                                nc.gpsimd.partition_all_reduce(
                                    ssq[:], sq[:], 128, ROP.add
                                )
                                std = p1.tile([128, TB], F32, tag="std")
                                nc.scalar.activation(
                                    std[:], ssq[:], AF.Sqrt,
                                    bias=eps_sb[:], scale=1.0 / D,
                                )
                                rec = p1.tile([128, TB], F32, tag="rec")
                                nc.vector.reciprocal(rec[:], std[:])
                                scl = qgs[:, g:g + 1] if g < 2 else None
                                qtmp = p1.tile([ROPE_DIMS, TB], F32, tag="qtmp")
                                if scl is not None:
                                    nc.vector.scalar_tensor_tensor(
                                        dest[ROPE_DIMS:128, colr],
                                        ps[ROPE_DIMS:128, :],
                                        scl[ROPE_DIMS:128, :],
                                        rec[ROPE_DIMS:128, :],
                                        ALU.mult, ALU.mult,
                                    )
                                    nc.vector.scalar_tensor_tensor(
                                        qtmp[:], ps[0:ROPE_DIMS, :],
                                        scl[0:ROPE_DIMS, :], rec[0:ROPE_DIMS, :],
                                        ALU.mult, ALU.mult,
                                    )
                                else:
                                    nc.vector.tensor_mul(
                                        dest[ROPE_DIMS:128, colr],
                                        ps[ROPE_DIMS:128, :], rec[ROPE_DIMS:128, :],
                                    )
                                    nc.vector.tensor_mul(
                                        qtmp[:], ps[0:ROPE_DIMS, :], rec[0:ROPE_DIMS, :]
                                    )
                                qsh = p1.tile([ROPE_DIMS, TB], F32, tag="qsh")
                                nc.sync.dma_start(qsh[0:32, :], qtmp[32:64, :])
                                nc.sync.dma_start(qsh[32:64, :], qtmp[0:32, :])
                                t64 = p1.tile([ROPE_DIMS, TB], F32, tag="t64")
                                u64 = p1.tile([ROPE_DIMS, TB], F32, tag="u64")
                                nc.vector.tensor_mul(t64[:], qtmp[:], cc_sb[:, posr])
                                nc.vector.tensor_mul(u64[:], qsh[:], ss_sb[:, posr])
                                nc.vector.tensor_add(
                                    dest[0:ROPE_DIMS, colr], t64[:], u64[:]
                                )
                            else:
                                # ---- v: raw out, transpose, residual mix ----
                                vr = p1.tile([128, TB], F32, tag="vr")
                                nc.scalar.copy(vr[:], ps[:])
                                nc.sync.dma_start(rawvT[:, colr], vr[:])
                                for j in range(TB // 128):
                                    ch = tok0 // 128 + j
                                    pt = ps1t.tile([128, 128], F32, tag="vtr")
                                    nc.tensor.transpose(
                                        pt[:], vr[:, j * 128:(j + 1) * 128], id_sb[:]
                                    )
                                    v0t = p1.tile([128, D], F32, tag="v0t")
                                    nc.sync.dma_start(
                                        v0t[:],
                                        v0c[tok0 + j * 128: tok0 + (j + 1) * 128, :],
                                    )
                                    v0s = p1.tile([128, D], F32, tag="v0s")
                                    nc.vector.tensor_scalar_mul(v0s[:], v0t[:], lam0)
                                    nc.vector.scalar_tensor_tensor(
                                        vmix[:, ch * 128:(ch + 1) * 128],
                                        pt[:], lam1, v0s[:], ALU.mult, ALU.add,
                                    )

                # ================= P2: causal attention =================
                with (
                    tc.tile_pool(name="p2", bufs=3) as p2,
                    tc.tile_pool(name="ps2s", bufs=2, space="PSUM") as ps2s,
                    tc.tile_pool(name="ps2y", bufs=2, space="PSUM") as ps2y,
                ):
                    for b in range(B):
                        for h in range(2):
                            for qb in range(NQB):
                                q0 = qb * QB
                                nk = (q0 + QB) // 128
                                psy = ps2y.tile([128, QB], F32, tag="psy")
                                sbar = p2.tile([128, QB], F32, tag="sbar", bufs=2)
                                for kc_ in range(nk):
                                    diag = kc_ * 128 >= q0
                                    u = kc_ * 128 - q0 if diag else 0
                                    fl = QB - u
                                    pss = ps2s.tile([128, QB], F32, tag="pss")
                                    nc.tensor.matmul(
                                        pss[:, 0:fl],
                                        kT[:, b * S + kc_ * 128:
                                           b * S + (kc_ + 1) * 128],
                                        qT[h][:, b * S + q0 + u: b * S + q0 + QB],
                                        start=True, stop=True,
                                    )
                                    ptl = p2.tile([128, QB], F32R, tag="ptl", bufs=3)
                                    nc.scalar.activation(
                                        ptl[:, 0:fl], pss[:, 0:fl], AF.Exp
                                    )
                                    if diag:
                                        nc.vector.tensor_mul(
                                            ptl[:, 0:128], ptl[:, 0:128], mtri_sb[:]
                                        )
                                    if kc_ == 0:
                                        nc.vector.tensor_copy(sbar[:], ptl[:])
                                    else:
                                        nc.vector.tensor_add(
                                            sbar[:, u:QB], sbar[:, u:QB], ptl[:, 0:fl]
                                        )
                                    nc.tensor.matmul(
                                        psy[:, u:QB],
                                        vmix[:, (b * S // 128 + kc_) * 128:
                                             (b * S // 128 + kc_ + 1) * 128],
                                        ptl[:, 0:fl],
                                        start=(kc_ == 0), stop=(kc_ == nk - 1),
                                    )
                                dbc = p2.tile([128, QB], F32, tag="dbc", bufs=2)
                                nc.gpsimd.partition_all_reduce(
                                    dbc[:], sbar[:], 128, ROP.add
                                )
                                # scatter yT + den straight into the A2A in-buffer
                                njq = QB // TSLICE
                                j0 = (b * S + q0) // TSLICE
                                yst = p2.tile([128, QB], F32, tag="yst", bufs=2)
                                nc.vector.tensor_copy(yst[:], psy[:])
                                nc.sync.dma_start(
                                    agi_r[h * D:(h + 1) * D, j0:j0 + njq, :],
                                    yst[:].rearrange("p (j c) -> p j c", c=TSLICE),
                                )
                                nc.sync.dma_start(
                                    agi_r[2 * D + h:2 * D + h + 1, j0:j0 + njq, :],
                                    dbc[0:1, :].rearrange("p (j c) -> p j c", c=TSLICE),
                                )

            # ================= P3: AllToAll reshard =================
            nc.gpsimd.collective_compute(
                "AllToAll",
                ALU.bypass,
                replica_groups=[list(range(NCORES))],
                ins=[agi.opt()],
                outs=[ago.opt()],
            )
            ago_r = ago[:].rearrange("(i r) c -> i r c", i=NCORES)  # [8,ROWS,TS]

            # ====== P4: gate + normalize + P5: out-proj (my token slice) =====
            with (
                tc.tile_pool(name="p4", bufs=1) as p4,
                tc.tile_pool(name="ps4", bufs=2, space="PSUM") as ps4,
                tc.tile_pool(name="p5", bufs=1) as p5,
                tc.tile_pool(name="ps5", bufs=4, space="PSUM") as ps5,
            ):
                gsig = p4.tile([H, TSLICE], F32)
                for tk in range(NT):
                    xg = p4.tile([128, 16, 128], F32R, tag="xg", bufs=2)
                    nc.sync.dma_start(
                        xg[:], gxT_r[:, :, tk * 128:(tk + 1) * 128].bitcast(F32R)
                    )
                    psg = ps4.tile([H, 128], F32, tag="psg")
                    for dc in range(16):
                        nc.tensor.matmul(
                            psg[:], gwT_sb[:, dc, :], xg[:, dc, :],
                            start=(dc == 0), stop=(dc == 15),
                        )
                    nc.scalar.activation(
                        gsig[:, tk * 128:(tk + 1) * 128], psg[:],
                        AF.Sigmoid, bias=gb_sb[:],
                    )
                den16 = p4.tile([H, TSLICE], F32)
                nc.sync.dma_start(den16[:], ago_r[:, 2 * D:2 * D + 2, :])
                rec16 = p4.tile([H, TSLICE], F32)
                nc.vector.reciprocal(rec16[:], den16[:])
                sc16 = p4.tile([H, TSLICE], F32)
                nc.vector.tensor_mul(sc16[:], gsig[:], rec16[:])
                ynorm = []
                for hh in range(H):
                    scrow = p4.tile([1, TSLICE], F32, tag="scrow", bufs=2)
                    nc.sync.dma_start(scrow[:], sc16[hh:hh + 1, :])
                    scb = p4.tile([128, TSLICE], F32, tag="scb", bufs=2)
                    nc.gpsimd.partition_broadcast(scb[:], scrow[:], 128)
                    yh = p4.tile([128, TSLICE], F32, tag="yh", bufs=2)
                    nc.sync.dma_start(
                        yh[:], ago_r[hh // 2, (hh % 2) * D:(hh % 2 + 1) * D, :]
                    )
                    yn = p4.tile([128, TSLICE], F32R, name=f"yn{hh}")
                    nc.vector.tensor_mul(yn[:], yh[:], scb[:])
                    ynorm.append(yn)

                wtiles = []
                for dv in range(H):
                    wt = p5.tile([128, DIM], F32R, name=f"wp{dv}")
                    nc.sync.dma_start(wt[:], wprojT_r[:, dv, :].bitcast(F32R))
                    wtiles.append(wt)
                for tk in range(NT):
                    for ec in range(NEC):
                        po = ps5.tile([128, 512], F32, tag="po")
                        for dv in range(H):
                            nc.tensor.matmul(
                                po[:],
                                ynorm[dv][:, tk * 128:(tk + 1) * 128],
                                wtiles[dv][:, ec * 512:(ec + 1) * 512],
                                start=(dv == 0), stop=(dv == H - 1),
                            )
                        ob = p4.tile([128, 512], F32, tag="ob", bufs=3)
                        nc.scalar.copy(ob[:], po[:])
                        nc.sync.dma_start(
                            outslice[tk * 128:(tk + 1) * 128,
                                     ec * 512:(ec + 1) * 512],
                            ob[:],
                        )

    nc.finalize()
    return nc


def _get_program(S):
    if S not in _BUILD_CACHE:
        _BUILD_CACHE[S] = build(S)
    return _BUILD_CACHE[S]


def _host_constants(S):
    inv = ROPE_BASE ** (-np.arange(0, ROPE_DIMS, 2, dtype=np.float32) / ROPE_DIMS)
    t = np.arange(S, dtype=np.float32)
    f = np.outer(t, inv)                       # [S, 32]
    cos = np.cos(f).T.astype(np.float32)       # [32, S]
    sin = np.sin(f).T.astype(np.float32)
    cc = np.ascontiguousarray(np.concatenate([cos, cos], axis=0))
    ss = np.ascontiguousarray(np.concatenate([sin, -sin], axis=0))
    i_ = np.arange(128)
    mtri = (i_[:, None] <= i_[None, :]).astype(np.float32)
    ident = np.eye(128, dtype=np.float32)
    return cc, ss, mtri, ident


def kernel(x, v0, Wq, Wk, Wv, Wproj, q_gain, vr_lambda, gate_w, gate_b):
    x = np.asarray(x, dtype=np.float32)
    v0 = np.asarray(v0, dtype=np.float32)
    Wq = np.asarray(Wq, dtype=np.float32)
    Wk = np.asarray(Wk, dtype=np.float32)
    Wv = np.asarray(Wv, dtype=np.float32)
    Wproj = np.asarray(Wproj, dtype=np.float32)
    q_gain = np.asarray(q_gain, dtype=np.float32)
    vr_lambda = np.asarray(vr_lambda, dtype=np.float32)
    gate_w = np.asarray(gate_w, dtype=np.float32)
    gate_b = np.asarray(gate_b, dtype=np.float32)

    Bx, S, _ = x.shape
    N = Bx * S
    TSLICE = N // NCORES
    nc = _get_program(S)
    cc, ss, mtri, ident = _host_constants(S)

    xT = np.ascontiguousarray(x.reshape(N, DIM).T)           # [DIM, N]
    wprojT = np.ascontiguousarray(Wproj.T)                   # [DIM, DIM]
    gwT = np.ascontiguousarray(gate_w.T)                     # [DIM, H]

    in_maps = []
    for c in range(NCORES):
        kc = c // 2
        wall = np.ascontiguousarray(np.concatenate(
            [
                Wq[2 * c * D:(2 * c + 2) * D, :].T,          # q0,q1 cols
                Wk[kc * D:(kc + 1) * D, :].T,                # k
                Wv[kc * D:(kc + 1) * D, :].T,                # v
                gate_w[2 * c:2 * c + 2, :].T,                # (pad cols)
            ],
            axis=1,
        ))
        v0cc = np.ascontiguousarray(v0[:, kc].reshape(N, D))
        xTg = np.ascontiguousarray(xT[:, c * TSLICE:(c + 1) * TSLICE])
        scal = np.ascontiguousarray(np.concatenate(
            [q_gain[2 * c:2 * c + 2], vr_lambda, gate_b]
        ).astype(np.float32))
        in_maps.append({
            "xT": xT, "wall": wall, "wprojT": wprojT, "gwT": gwT,
            "v0c": v0cc, "xTg": xTg, "cc": cc, "ss": ss,
            "mtri": mtri, "ident": ident, "scal": scal,
        })

    res = run_bass_kernel_spmd(nc, in_maps, list(range(NCORES)))

    out = np.concatenate(
        [res.results[c]["outslice"] for c in range(NCORES)], axis=0
    ).reshape(Bx, S, DIM)
    raw_v = np.stack(
        [res.results[2 * kc]["rawvT"].T.reshape(Bx, S, D) for kc in range(HKV)],
        axis=1,
    )
    return (out, raw_v)


# revision 11
# speedup vs baseline: 1.0069x; 1.0069x over previous
"""Trainium2 Bass kernel for nn_CausalSelfAttention_88364657148333.

8-core SPMD: tensor-parallel over heads (2 q-heads + 1 kv-head per core),
AllToAll reshard before the output projection (each core finishes 1/8 of
the tokens). All matmuls run in float32r (full-rate fp32 on the PE).
"""

import math
import sys

import numpy as np

try:
    import concourse.bass  # noqa: F401
except ImportError:
    sys.path.insert(0, "/opt/trn_rl_repo")

import concourse.bass_isa as bass_isa  # noqa: E402
import concourse.mybir as mybir  # noqa: E402
import concourse.tile as tile  # noqa: E402
from concourse import bacc  # noqa: E402
from concourse.bass_utils import run_bass_kernel_spmd  # noqa: E402

F32 = mybir.dt.float32
F32R = mybir.dt.float32r
AF = mybir.ActivationFunctionType
ALU = mybir.AluOpType
ROP = bass_isa.ReduceOp

B = 2
DIM = 2048
H = 16
HKV = 4
D = 128
NCORES = 8
EPS = 1.1920929e-07
ROPE_DIMS = 64
ROPE_BASE = 10000.0

_BUILD_CACHE: dict = {}


def build(S: int = 2048):
    """Build the SPMD Bass program (identical on all 8 cores)."""
    N = B * S                 # total tokens
    TSLICE = N // NCORES      # output tokens per core
    TB = 256                  # projection token block
    NTB = N // TB
    QB = min(512, S)          # attention query block (within one batch)
    NQB = S // QB
    NT = TSLICE // 128        # out-proj token chunks per core
    NEC = DIM // 512          # out-proj feature chunks
    ROWS = 2 * D + 2          # A2A bundle rows per core (yT h0, yT h1, den)
    assert S % 512 == 0 and TSLICE % 128 == 0 and QB % TSLICE == 0

    nc = bacc.Bacc("TRN2", target_bir_lowering=False, num_devices=NCORES)

    # ---- DRAM I/O (per-core contents prepared by kernel() on the host) ----
    xT = nc.dram_tensor("xT", [DIM, N], F32, kind="ExternalInput")
    wall = nc.dram_tensor("wall", [DIM, 514], F32, kind="ExternalInput")
    wprojT = nc.dram_tensor("wprojT", [DIM, DIM], F32, kind="ExternalInput")
    gwT = nc.dram_tensor("gwT", [DIM, H], F32, kind="ExternalInput")
    v0c = nc.dram_tensor("v0c", [N, D], F32, kind="ExternalInput")
    xTg = nc.dram_tensor("xTg", [DIM, TSLICE], F32, kind="ExternalInput")
    cc = nc.dram_tensor("cc", [ROPE_DIMS, S], F32, kind="ExternalInput")
    ss = nc.dram_tensor("ss", [ROPE_DIMS, S], F32, kind="ExternalInput")
    mtri = nc.dram_tensor("mtri", [128, 128], F32, kind="ExternalInput")
    ident = nc.dram_tensor("ident", [128, 128], F32, kind="ExternalInput")
    scal = nc.dram_tensor("scal", [4 + H], F32, kind="ExternalInput")

    outslice = nc.dram_tensor("outslice", [TSLICE, DIM], F32, kind="ExternalOutput")
    rawvT = nc.dram_tensor("rawvT", [D, N], F32, kind="ExternalOutput")

    xT_r = xT.ap().rearrange("(c p) n -> p c n", p=128)        # [128,16,N]
    wall_r = wall.ap().rearrange("(c p) m -> p c m", p=128)    # [128,16,514]
    wprojT_r = wprojT.ap().rearrange("(c p) e -> p c e", p=128)
    gwT_r = gwT.ap().rearrange("(c p) h -> p c h", p=128)
    gxT_r = xTg.ap().rearrange("(c p) n -> p c n", p=128)

    with tile.TileContext(nc) as tc:
        with (
            tc.tile_pool(name="consts", bufs=1) as consts,
            tc.tile_pool(name="dram", bufs=1, space="DRAM") as dram,
        ):
            # ---------- small constants ----------
            mtri_sb = consts.tile([128, 128], F32)
            id_sb = consts.tile([128, 128], F32)
            gwT_sb = consts.tile([128, 16, H], F32R)
            scal_row = consts.tile([1, 4 + H], F32)
            scal_b = consts.tile([128, 4], F32)
            qgs = consts.tile([128, 2], F32)
            gb_sb = consts.tile([H, 1], F32)
            eps_sb = consts.tile([128, 1], F32)
            nc.vector.memset(eps_sb[:], EPS)
            nc.sync.dma_start(mtri_sb[:], mtri[:, :])
            nc.sync.dma_start(id_sb[:], ident[:, :])
            nc.sync.dma_start(gwT_sb[:], gwT_r.bitcast(F32R))
            nc.sync.dma_start(scal_row[:], scal[None, :])
            nc.sync.dma_start(gb_sb[:], scal[4:4 + H, None])
            nc.gpsimd.partition_broadcast(scal_b[:], scal_row[:, 0:4], 128)
            nc.vector.tensor_scalar_mul(qgs[:], scal_b[:, 0:2], 1.0 / math.sqrt(D))
            lam0 = scal_b[:, 2:3]
            lam1 = scal_b[:, 3:4]

            # ---------- A2A bundle buffers (DRAM) ----------
            agi = dram.tile([NCORES * ROWS, TSLICE], F32)
            ago = dram.tile([NCORES * ROWS, TSLICE], F32)
            agi_r = agi[:].rearrange("(j r) c -> r j c", j=NCORES)  # [ROWS,8,TS]

            # ---------- P1+P2, one batch at a time ----------
            with tc.tile_pool(name="p1w", bufs=1) as p1w:
                wall_sb = p1w.tile([128, 16, 514], F32R)
                nc.sync.dma_start(wall_sb[:], wall_r.bitcast(F32R))
                for b in range(B):
                    bs = b * S
                    with tc.tile_pool(name=f"persist{b}", bufs=1) as persist2:
                        qT = [persist2.tile([128, S], F32R, name=f"qT{h}_{b}")
                              for h in range(2)]
                        kT = persist2.tile([128, S], F32R, name=f"kT_{b}")
                        vmix = persist2.tile([128, S], F32R, name=f"vmix_{b}")

                        # ---- P1(b): projections, 512-token blocks ----
                        with (
                            tc.tile_pool(name=f"p1_{b}", bufs=2) as p1,
                            tc.tile_pool(name=f"ps1_{b}", bufs=3, space="PSUM") as ps1,
                            tc.tile_pool(name=f"ps1t_{b}", bufs=2, space="PSUM") as ps1t,
                        ):
                            for tb in range(S // 512):
                                tok0 = tb * 512
                                colr = slice(tok0, tok0 + 512)
                                xt = p1.tile([128, 16, 512], F32R, tag="xt", bufs=2)
                                nc.sync.dma_start(
                                    xt[:],
                                    xT_r[:, :, bs + tok0: bs + tok0 + 512].bitcast(F32R),
                                )
                                csc = p1.tile([ROPE_DIMS, 512], F32, tag="csc", bufs=2)
                                css = p1.tile([ROPE_DIMS, 512], F32, tag="css", bufs=2)
                                nc.sync.dma_start(csc[:], cc[:, colr])
                                nc.sync.dma_start(css[:], ss[:, colr])
                                for g in range(4):
                                    ps = ps1.tile([128, 512], F32, tag="proj")
                                    for dc in range(16):
                                        nc.tensor.matmul(
                                            ps[:],
                                            wall_sb[:, dc, g * 128:(g + 1) * 128],
                                            xt[:, dc, :],
                                            start=(dc == 0),
                                            stop=(dc == 15),
                                        )
                                    if g < 3:
                                        # ---- RMS norm + partial RoPE ----
                                        dest = qT[g] if g < 2 else kT
                                        sq = p1.tile([128, 512], F32, tag="sq")
                                        nc.scalar.square(sq[:], ps[:])
                                        ssq = p1.tile([128, 512], F32, tag="ssq")
                                        nc.gpsimd.partition_all_reduce(
                                            ssq[:], sq[:], 128, ROP.add
                                        )
                                        std = p1.tile([128, 512], F32, tag="std")
                                        nc.scalar.activation(
                                            std[:], ssq[:], AF.Sqrt,
                                            bias=eps_sb[:], scale=1.0 / D,
                                        )
                                        rec = p1.tile([128, 512], F32, tag="rec")
                                        nc.vector.reciprocal(rec[:], std[:])
                                        scl = qgs[:, g:g + 1] if g < 2 else None
                                        qtmp = p1.tile([ROPE_DIMS, 512], F32, tag="qtmp")
                                        if scl is not None:
                                            nc.vector.scalar_tensor_tensor(
                                                dest[ROPE_DIMS:128, colr],
                                                ps[ROPE_DIMS:128, :],
                                                scl[ROPE_DIMS:128, :],
                                                rec[ROPE_DIMS:128, :],
                                                ALU.mult, ALU.mult,
                                            )
                                            nc.vector.scalar_tensor_tensor(
                                                qtmp[:], ps[0:ROPE_DIMS, :],
                                                scl[0:ROPE_DIMS, :],
                                                rec[0:ROPE_DIMS, :],
                                                ALU.mult, ALU.mult,
                                            )
                                        else:
                                            nc.vector.tensor_mul(
                                                dest[ROPE_DIMS:128, colr],
                                                ps[ROPE_DIMS:128, :],
                                                rec[ROPE_DIMS:128, :],
                                            )
                                            nc.vector.tensor_mul(
                                                qtmp[:], ps[0:ROPE_DIMS, :],
                                                rec[0:ROPE_DIMS, :],
                                            )
                                        qsh = p1.tile([ROPE_DIMS, 512], F32, tag="qsh")
                                        nc.sync.dma_start(qsh[0:32, :], qtmp[32:64, :])
                                        nc.sync.dma_start(qsh[32:64, :], qtmp[0:32, :])
                                        t64 = p1.tile([ROPE_DIMS, 512], F32, tag="t64")
                                        u64 = p1.tile([ROPE_DIMS, 512], F32, tag="u64")
                                        nc.vector.tensor_mul(t64[:], qtmp[:], csc[:])
                                        nc.vector.tensor_mul(u64[:], qsh[:], css[:])
                                        nc.vector.tensor_add(
                                            dest[0:ROPE_DIMS, colr], t64[:], u64[:]
                                        )
                                    else:
                                        # ---- v: raw out, transpose, mix ----
                                        vr = p1.tile([128, 512], F32, tag="vr")
                                        nc.scalar.copy(vr[:], ps[:])
                                        nc.sync.dma_start(
                                            rawvT[:, bs + tok0: bs + tok0 + 512], vr[:]
                                        )
                                        for j in range(4):
                                            ch = tok0 // 128 + j
                                            pt = ps1t.tile([128, 128], F32, tag="vtr")
                                            nc.tensor.transpose(
                                                pt[:],
                                                vr[:, j * 128:(j + 1) * 128], id_sb[:],
                                            )
                                            v0t = p1.tile([128, D], F32, tag="v0t")
                                            nc.sync.dma_start(
                                                v0t[:],
                                                v0c[bs + tok0 + j * 128:
                                                    bs + tok0 + (j + 1) * 128, :],
                                            )
                                            v0s = p1.tile([128, D], F32, tag="v0s")
                                            nc.vector.tensor_scalar_mul(
                                                v0s[:], v0t[:], lam0
                                            )
                                            nc.vector.scalar_tensor_tensor(
                                                vmix[:, ch * 128:(ch + 1) * 128],
                                                pt[:], lam1, v0s[:],
                                                ALU.mult, ALU.add,
                                            )

                        # ---- P2(b): causal attention ----
                        with (
                            tc.tile_pool(name=f"p2_{b}", bufs=3) as p2,
                            tc.tile_pool(name=f"ps2s_{b}", bufs=2, space="PSUM") as ps2s,
                            tc.tile_pool(name=f"ps2y_{b}", bufs=2, space="PSUM") as ps2y,
                        ):
                            for h in range(2):
                                for qb in range(NQB):
                                    q0 = qb * QB
                                    nk = (q0 + QB) // 128
                                    psy = ps2y.tile([128, QB], F32, tag="psy")
                                    sbar = p2.tile([128, QB], F32, tag="sbar", bufs=2)
                                    for kc_ in range(nk):
                                        diag = kc_ * 128 >= q0
                                        u = kc_ * 128 - q0 if diag else 0
                                        fl = QB - u
                                        pss = ps2s.tile([128, QB], F32, tag="pss")
                                        nc.tensor.matmul(
                                            pss[:, 0:fl],
                                            kT[:, kc_ * 128:(kc_ + 1) * 128],
                                            qT[h][:, q0 + u: q0 + QB],
                                            start=True, stop=True,
                                        )
                                        ptl = p2.tile([128, QB], F32R, tag="ptl", bufs=3)
                                        nc.scalar.activation(
                                            ptl[:, 0:fl], pss[:, 0:fl], AF.Exp
                                        )
                                        if diag:
                                            nc.vector.tensor_mul(
                                                ptl[:, 0:128], ptl[:, 0:128], mtri_sb[:]
                                            )
                                        if kc_ == 0:
                                            nc.vector.tensor_copy(sbar[:], ptl[:])
                                        else:
                                            nc.vector.tensor_add(
                                                sbar[:, u:QB], sbar[:, u:QB],
                                                ptl[:, 0:fl],
                                            )
                                        nc.tensor.matmul(
                                            psy[:, u:QB],
                                            vmix[:, kc_ * 128:(kc_ + 1) * 128],
                                            ptl[:, 0:fl],
                                            start=(kc_ == 0), stop=(kc_ == nk - 1),
                                        )
                                    dbc = p2.tile([128, QB], F32, tag="dbc", bufs=2)
                                    nc.gpsimd.partition_all_reduce(
                                        dbc[:], sbar[:], 128, ROP.add
                                    )
                                    njq = QB // TSLICE
                                    j0 = (bs + q0) // TSLICE
                                    yst = p2.tile([128, QB], F32, tag="yst", bufs=2)
                                    nc.vector.tensor_copy(yst[:], psy[:])
                                    nc.sync.dma_start(
                                        agi_r[h * D:(h + 1) * D, j0:j0 + njq, :],
                                        yst[:].rearrange("p (j c) -> p j c", c=TSLICE),
                                    )
                                    nc.sync.dma_start(
                                        agi_r[2 * D + h:2 * D + h + 1, j0:j0 + njq, :],
                                        dbc[0:1, :].rearrange(
                                            "p (j c) -> p j c", c=TSLICE
                                        ),
                                    )

            # ================= P3: AllToAll reshard =================
            nc.gpsimd.collective_compute(
                "AllToAll",
                ALU.bypass,
                replica_groups=[list(range(NCORES))],
                ins=[agi.opt()],
                outs=[ago.opt()],
            )
            ago_r = ago[:].rearrange("(i r) c -> i r c", i=NCORES)  # [8,ROWS,TS]

            # ====== P4: gate + normalize + P5: out-proj (my token slice) =====
            with (
                tc.tile_pool(name="p4", bufs=1) as p4,
                tc.tile_pool(name="ps4", bufs=2, space="PSUM") as ps4,
                tc.tile_pool(name="p5", bufs=1) as p5,
                tc.tile_pool(name="ps5", bufs=4, space="PSUM") as ps5,
            ):
                gsig = p4.tile([H, TSLICE], F32)
                for tk in range(NT):
                    xg = p4.tile([128, 16, 128], F32R, tag="xg", bufs=2)
                    nc.sync.dma_start(
                        xg[:], gxT_r[:, :, tk * 128:(tk + 1) * 128].bitcast(F32R)
                    )
                    psg = ps4.tile([H, 128], F32, tag="psg")
                    for dc in range(16):
                        nc.tensor.matmul(
                            psg[:], gwT_sb[:, dc, :], xg[:, dc, :],
                            start=(dc == 0), stop=(dc == 15),
                        )
                    nc.scalar.activation(
                        gsig[:, tk * 128:(tk + 1) * 128], psg[:],
                        AF.Sigmoid, bias=gb_sb[:],
                    )
                den16 = p4.tile([H, TSLICE], F32)
                nc.sync.dma_start(den16[:], ago_r[:, 2 * D:2 * D + 2, :])
                rec16 = p4.tile([H, TSLICE], F32)
                nc.vector.reciprocal(rec16[:], den16[:])
                sc16 = p4.tile([H, TSLICE], F32)
                nc.vector.tensor_mul(sc16[:], gsig[:], rec16[:])
                ynorm = []
                for hh in range(H):
                    scrow = p4.tile([1, TSLICE], F32, tag="scrow", bufs=2)
                    nc.sync.dma_start(scrow[:], sc16[hh:hh + 1, :])
                    scb = p4.tile([128, TSLICE], F32, tag="scb", bufs=2)
                    nc.gpsimd.partition_broadcast(scb[:], scrow[:], 128)
                    yh = p4.tile([128, TSLICE], F32, tag="yh", bufs=2)
                    nc.sync.dma_start(
                        yh[:], ago_r[hh // 2, (hh % 2) * D:(hh % 2 + 1) * D, :]
                    )
                    yn = p4.tile([128, TSLICE], F32R, name=f"yn{hh}")
                    nc.vector.tensor_mul(yn[:], yh[:], scb[:])
                    ynorm.append(yn)

                wtiles = []
                for dv in range(H):
                    wt = p5.tile([128, DIM], F32R, name=f"wp{dv}")
                    nc.sync.dma_start(wt[:], wprojT_r[:, dv, :].bitcast(F32R))
                    wtiles.append(wt)
                for tk in range(NT):
                    for ec in range(NEC):
                        po = ps5.tile([128, 512], F32, tag="po")
                        for dv in range(H):
                            nc.tensor.matmul(
                                po[:],
                                ynorm[dv][:, tk * 128:(tk + 1) * 128],
                                wtiles[dv][:, ec * 512:(ec + 1) * 512],
                                start=(dv == 0), stop=(dv == H - 1),
                            )
                        ob = p4.tile([128, 512], F32, tag="ob", bufs=3)
                        nc.scalar.copy(ob[:], po[:])
                        nc.sync.dma_start(
                            outslice[tk * 128:(tk + 1) * 128,
                                     ec * 512:(ec + 1) * 512],
                            ob[:],
                        )

    nc.finalize()
    return nc


def _get_program(S):
    if S not in _BUILD_CACHE:
        _BUILD_CACHE[S] = build(S)
    return _BUILD_CACHE[S]


def _host_constants(S):
    inv = ROPE_BASE ** (-np.arange(0, ROPE_DIMS, 2, dtype=np.float32) / ROPE_DIMS)
    t = np.arange(S, dtype=np.float32)
    f = np.outer(t, inv)                       # [S, 32]
    cos = np.cos(f).T.astype(np.float32)       # [32, S]
    sin = np.sin(f).T.astype(np.float32)
    cc = np.ascontiguousarray(np.concatenate([cos, cos], axis=0))
    ss = np.ascontiguousarray(np.concatenate([sin, -sin], axis=0))
    i_ = np.arange(128)
    mtri = (i_[:, None] <= i_[None, :]).astype(np.float32)
    ident = np.eye(128, dtype=np.float32)
    return cc, ss, mtri, ident


def kernel(x, v0, Wq, Wk, Wv, Wproj, q_gain, vr_lambda, gate_w, gate_b):
    x = np.asarray(x, dtype=np.float32)
    v0 = np.asarray(v0, dtype=np.float32)
    Wq = np.asarray(Wq, dtype=np.float32)
    Wk = np.asarray(Wk, dtype=np.float32)
    Wv = np.asarray(Wv, dtype=np.float32)
    Wproj = np.asarray(Wproj, dtype=np.float32)
    q_gain = np.asarray(q_gain, dtype=np.float32)
    vr_lambda = np.asarray(vr_lambda, dtype=np.float32)
    gate_w = np.asarray(gate_w, dtype=np.float32)
    gate_b = np.asarray(gate_b, dtype=np.float32)

    Bx, S, _ = x.shape
    N = Bx * S
    TSLICE = N // NCORES
    nc = _get_program(S)
    cc, ss, mtri, ident = _host_constants(S)

    xT = np.ascontiguousarray(x.reshape(N, DIM).T)           # [DIM, N]
    wprojT = np.ascontiguousarray(Wproj.T)                   # [DIM, DIM]
    gwT = np.ascontiguousarray(gate_w.T)                     # [DIM, H]

    in_maps = []
    for c in range(NCORES):
        kc = c // 2
        wall = np.ascontiguousarray(np.concatenate(
            [
                Wq[2 * c * D:(2 * c + 2) * D, :].T,          # q0,q1 cols
                Wk[kc * D:(kc + 1) * D, :].T,                # k
                Wv[kc * D:(kc + 1) * D, :].T,                # v
                gate_w[2 * c:2 * c + 2, :].T,                # (pad cols)
            ],
            axis=1,
        ))
        v0cc = np.ascontiguousarray(v0[:, kc].reshape(N, D))
        xTg = np.ascontiguousarray(xT[:, c * TSLICE:(c + 1) * TSLICE])
        scal = np.ascontiguousarray(np.concatenate(
            [q_gain[2 * c:2 * c + 2], vr_lambda, gate_b]
        ).astype(np.float32))
        in_maps.append({
            "xT": xT, "wall": wall, "wprojT": wprojT, "gwT": gwT,
            "v0c": v0cc, "xTg": xTg, "cc": cc, "ss": ss,
            "mtri": mtri, "ident": ident, "scal": scal,
        })

    res = run_bass_kernel_spmd(nc, in_maps, list(range(NCORES)))

    out = np.concatenate(
        [res.results[c]["outslice"] for c in range(NCORES)], axis=0
    ).reshape(Bx, S, DIM)
    raw_v = np.stack(
        [res.results[2 * kc]["rawvT"].T.reshape(Bx, S, D) for kc in range(HKV)],
        axis=1,
    )
    return (out, raw_v)
